# revision 16
# baseline (speedup 1.0000x reference)
"""Expert-parallel MoE layer for Trainium2 (8 NeuronCores, 1 expert per core).

Fully static kernel (no dynamic control flow, no gpsimd ucode — neither is
available in this runtime):

Per core e:
  gating:   logitsT = Wg.T @ xT (fp32 matmuls) + bg; softmax + top-2 with
            DVE/ACT ops (fp32 — top-2 selection is tie-sensitive)
  routing:  mask for expert e -> exclusive prefix sum (PE matmul with a
            host-provided triangular-ones matrix) -> compacted slot per
            token -> indirect-DMA scatter of (token_id, gate) rows into a
            DRAM scratch -> read back the compacted list
  ffn:      fixed 1280-token capacity (counts are ~1024±28), slices of
            512/512/256 tokens: indirect-DMA gather -> PE transpose ->
            FFN1 (f32r) -> ReLU+b1 -> FFN2 (f32r) -> +b2 -> PE transpose
            -> scale by gate -> indirect-DMA scatter into partial output
            [N+1, D] (row N is a trash row)
Host: shards W1/b1/W2/b2 by expert, replicates x/xT/Wg, sums the 8 partials.

Hardcoded for: N=4096 tokens, D=1024, H=4096, E=8 experts, top-k=2.
"""

import numpy as np
from contextlib import ExitStack

import concourse.bass as bass
import concourse.mybir as mybir
import concourse.tile as tile
from concourse import bacc
from concourse.bass import ds, IndirectOffsetOnAxis
from concourse.bass_utils import run_bass_kernel_spmd

N, D, H, E, TOPK = 4096, 1024, 4096, 8, 2
P = 128
NT = N // P                       # 32 token tiles
DT = D // P                       # 8 d-tiles
HT = H // P                       # 32 h-tiles
C_CAP = 1280                      # static routed-token capacity per expert
NTC = C_CAP // P                  # 10 capacity tiles
SLICES = [(0, 512), (4, 512), (8, 256)]   # (tile offset, tokens)
SCR_ROWS = 4224                   # compaction scratch rows; 4223 = trash
TRASH = SCR_ROWS - 1

f32 = mybir.dt.float32
f32r = mybir.dt.float32r
i32 = mybir.dt.int32
AF = mybir.ActivationFunctionType
ALU = mybir.AluOpType

_NC_CACHE = {}

TRACE = False
LAST = {}


def build():
    key = "moe_static"
    if key in _NC_CACHE:
        return _NC_CACHE[key]

    nc = bacc.Bacc("TRN2", target_bir_lowering=False, debug=False)

    x_d = nc.declare_dram_parameter("x", [N, D], f32, isOutput=False)
    xT_d = nc.declare_dram_parameter("xT", [D, N], f32, isOutput=False)
    wg_d = nc.declare_dram_parameter("Wg", [D, E], f32, isOutput=False)
    bgc_d = nc.declare_dram_parameter("bgc", [E, 1], f32, isOutput=False)
    w1_d = nc.declare_dram_parameter("W1e", [D, H], f32r, isOutput=False)
    b1_d = nc.declare_dram_parameter("b1e", [1, H], f32, isOutput=False)
    w2_d = nc.declare_dram_parameter("W2e", [H, D], f32r, isOutput=False)
    b2_d = nc.declare_dram_parameter("b2e", [1, D], f32, isOutput=False)
    eid_d = nc.declare_dram_parameter("eid", [P, 1], f32, isOutput=False)
    iota_d = nc.declare_dram_parameter("iota8", [P, 8], f32, isOutput=False)
    iotar_d = nc.declare_dram_parameter("iota8r", [P, 8], f32, isOutput=False)
    itok_d = nc.declare_dram_parameter("iotatok", [P, NT], f32, isOutput=False)
    ident_d = nc.declare_dram_parameter("ident", [P, P], f32, isOutput=False)
    ut_d = nc.declare_dram_parameter("utri", [P, P], f32, isOutput=False)
    ones1_d = nc.declare_dram_parameter("ones1", [1, P], f32, isOutput=False)
    onesc_d = nc.declare_dram_parameter("onescol", [P, 1], f32, isOutput=False)
    fill_d = nc.declare_dram_parameter("fillrow", [P, 8], f32, isOutput=False)

    out_d = nc.declare_dram_parameter("ypart", [N + 1, D], f32, isOutput=True)
    scr_d = nc.dram_tensor("scratch", [SCR_ROWS, 8], f32)

    with tile.TileContext(nc) as tc:
        with ExitStack() as ctx:
            cpool = ctx.enter_context(tc.tile_pool(name="consts", bufs=1))

            ident = cpool.tile([P, P], f32)
            nc.sync.dma_start(ident[:], ident_d[:])
            utri = cpool.tile([P, P], f32)
            nc.sync.dma_start(utri[:], ut_d[:])
            ones1 = cpool.tile([1, P], f32)
            nc.sync.dma_start(ones1[:], ones1_d[:])
            onescol = cpool.tile([P, 1], f32)
            nc.sync.dma_start(onescol[:], onesc_d[:])
            iota8 = cpool.tile([P, 8], f32)
            nc.sync.dma_start(iota8[:], iota_d[:])
            iota8r = cpool.tile([P, 8], f32)
            nc.sync.dma_start(iota8r[:], iotar_d[:])
            iotatok = cpool.tile([P, NT], f32)
            nc.sync.dma_start(iotatok[:], itok_d[:])
            eid = cpool.tile([P, 1], f32)
            nc.sync.dma_start(eid[:], eid_d[:])
            fillrow = cpool.tile([P, 8], f32)
            nc.sync.dma_start(fillrow[:], fill_d[:])
            bgc = cpool.tile([E, 1], f32)
            nc.sync.dma_start(bgc[:], bgc_d[:])
            wg_sb = cpool.tile([P, DT, E], f32)
            for dt_i in range(DT):
                nc.sync.dma_start(wg_sb[:, dt_i, :],
                                  wg_d[dt_i * P:(dt_i + 1) * P, :])
            b1_sb = cpool.tile([P, HT], f32)
            nc.sync.dma_start(b1_sb[:], b1_d[0, :].rearrange("(a p) -> p a", p=P))
            b2_sb = cpool.tile([P, DT], f32)
            nc.sync.dma_start(b2_sb[:], b2_d[0, :].rearrange("(a p) -> p a", p=P))

            zero_t = cpool.tile([P, D], f32)
            nc.vector.memset(zero_t[:], 0.0)

            # zero-fill partial output; prefill compaction scratch
            for blk in range(N // P):
                nc.sync.dma_start(out_d[blk * P:(blk + 1) * P, :], zero_t[:])
            nc.sync.dma_start(out_d[N:N + 1, :], zero_t[0:1, :])
            for t in range(NTC):
                nc.sync.dma_start(scr_d[t * P:(t + 1) * P, :], fillrow[:])

            # ================= gating =================
            logits = cpool.tile([P, NT, 8], f32)
            with ExitStack() as gctx:
                gx = gctx.enter_context(tc.tile_pool(name="gx", bufs=3))
                gps = gctx.enter_context(
                    tc.tile_pool(name="gps", bufs=2, space="PSUM"))
                gtp = gctx.enter_context(
                    tc.tile_pool(name="gtp", bufs=2, space="PSUM"))
                gsb = gctx.enter_context(tc.tile_pool(name="gsb", bufs=2))

                GCS = 512
                for s in range(N // GCS):
                    lg_ps = gps.tile([E, GCS], f32, tag="lg")
                    for dt_i in range(DT):
                        xt_t = gx.tile([P, GCS], f32, tag="xt")
                        nc.sync.dma_start(
                            xt_t[:],
                            xT_d[dt_i * P:(dt_i + 1) * P,
                                 s * GCS:(s + 1) * GCS])
                        nc.tensor.matmul(
                            lg_ps[:], wg_sb[:, dt_i, :], xt_t[:],
                            start=(dt_i == 0), stop=(dt_i == DT - 1))
                    lg_sb = gsb.tile([E, GCS], f32, tag="lgsb")
                    nc.scalar.activation(lg_sb[:], lg_ps[:], AF.Identity,
                                         bias=bgc[:])
                    for c in range(GCS // P):
                        tr_ps = gtp.tile([P, 8], f32, tag="tr")
                        nc.tensor.transpose(tr_ps[:],
                                            lg_sb[:, c * P:(c + 1) * P],
                                            ident[0:E, 0:E])
                        nc.vector.tensor_copy(logits[:, s * (GCS // P) + c, :],
                                              tr_ps[:])

            # softmax + top2 (token-major [128, NT, 8])
            gt = ctx.enter_context(tc.tile_pool(name="gt", bufs=1))
            negm = gt.tile([P, NT], f32)
            nc.vector.tensor_reduce(negm[:], logits[:], mybir.AxisListType.X,
                                    ALU.max, negate=True)
            e_sb = gt.tile([P, NT, 8], f32)
            for t in range(NT):
                nc.scalar.activation(e_sb[:, t, :], logits[:, t, :], AF.Exp,
                                     bias=negm[:, t:t + 1])
            ssum = gt.tile([P, NT], f32)
            nc.vector.tensor_reduce(ssum[:], e_sb[:], mybir.AxisListType.X,
                                    ALU.add)
            rsum = gt.tile([P, NT], f32)
            nc.vector.reciprocal(rsum[:], ssum[:])

            def argmax8(e_t, tag):
                mx = gt.tile([P, NT], f32, tag=tag + "mx", name=tag + "mx")
                nc.vector.tensor_reduce(mx[:], e_t[:], mybir.AxisListType.X,
                                        ALU.max)
                eq = gt.tile([P, NT, 8], f32, tag=tag + "eq", name=tag + "eq")
                nc.vector.tensor_tensor(
                    out=eq[:], in0=e_t[:],
                    in1=mx[:, :, None].to_broadcast([P, NT, 8]),
                    op=ALU.is_ge)
                rv = gt.tile([P, NT, 8], f32, tag=tag + "rv", name=tag + "rv")
                nc.vector.tensor_tensor(
                    out=rv[:], in0=eq[:],
                    in1=iota8r[:, None, :].to_broadcast([P, NT, 8]),
                    op=ALU.mult)
                im = gt.tile([P, NT], f32, tag=tag + "im", name=tag + "im")
                nc.vector.tensor_reduce(im[:], rv[:], mybir.AxisListType.X,
                                        ALU.max)
                idx = gt.tile([P, NT], f32, tag=tag + "idx", name=tag + "idx")
                nc.vector.tensor_scalar(idx[:], im[:], -1.0, 7.0,
                                        ALU.mult, ALU.add)
                return mx, idx

            e1, idx1 = argmax8(e_sb, "a1")
            mask1 = gt.tile([P, NT, 8], f32)
            nc.vector.tensor_tensor(
                out=mask1[:],
                in0=iota8[:, None, :].to_broadcast([P, NT, 8]),
                in1=idx1[:, :, None].to_broadcast([P, NT, 8]),
                op=ALU.is_equal)
            e_m = gt.tile([P, NT, 8], f32)
            nc.vector.tensor_tensor(out=e_m[:], in0=e_sb[:], in1=mask1[:],
                                    op=ALU.mult)
            e_rest = gt.tile([P, NT, 8], f32)
            nc.vector.tensor_tensor(out=e_rest[:], in0=e_sb[:], in1=e_m[:],
                                    op=ALU.subtract)
            e2, idx2 = argmax8(e_rest, "a2")

            # this expert's mask + gate value per token
            eq1 = gt.tile([P, NT], f32)
            nc.vector.tensor_scalar(eq1[:], idx1[:], eid[:, 0:1], None,
                                    ALU.is_equal)
            eq2 = gt.tile([P, NT], f32)
            nc.vector.tensor_scalar(eq2[:], idx2[:], eid[:, 0:1], None,
                                    ALU.is_equal)
            msk = gt.tile([P, NT], f32)
            nc.vector.tensor_tensor(out=msk[:], in0=eq1[:], in1=eq2[:],
                                    op=ALU.add)
            ge1 = gt.tile([P, NT], f32)
            nc.vector.tensor_tensor(out=ge1[:], in0=e1[:], in1=eq1[:],
                                    op=ALU.mult)
            ge2 = gt.tile([P, NT], f32)
            nc.vector.tensor_tensor(out=ge2[:], in0=e2[:], in1=eq2[:],
                                    op=ALU.mult)
            gsum = gt.tile([P, NT], f32)
            nc.vector.tensor_tensor(out=gsum[:], in0=ge1[:], in1=ge2[:],
                                    op=ALU.add)
            g_e = gt.tile([P, NT], f32)
            nc.vector.tensor_tensor(out=g_e[:], in0=gsum[:], in1=rsum[:],
                                    op=ALU.mult)

            # ============ compaction (prefix sum + scatter) ============
            with ExitStack() as pctx:
                pps = pctx.enter_context(
                    tc.tile_pool(name="pps", bufs=1, space="PSUM"))
                cum_ps = pps.tile([P, NT], f32)
                nc.tensor.matmul(cum_ps[:], utri[:], msk[:])
                cum = gt.tile([P, NT], f32)
                nc.vector.tensor_copy(cum[:], cum_ps[:])
                pos_in = gt.tile([P, NT], f32)
                nc.vector.tensor_tensor(out=pos_in[:], in0=cum[:], in1=msk[:],
                                        op=ALU.subtract)

                # exclusive scan of the 32 tile totals (log-shift adds)
                tot_ps = pps.tile([1, NT], f32, name="tot_ps")
                nc.tensor.matmul(tot_ps[:], onescol[:], msk[:])
                tot = gt.tile([1, NT], f32)
                nc.vector.tensor_copy(tot[:], tot_ps[:])
                sc_a = gt.tile([1, NT], f32)
                nc.vector.tensor_copy(sc_a[:], tot[:])
                sc_b = gt.tile([1, NT], f32)
                for sh in (1, 2, 4, 8, 16):
                    nc.vector.tensor_copy(sc_b[:, 0:sh], sc_a[:, 0:sh])
                    nc.vector.tensor_tensor(out=sc_b[:, sh:NT],
                                            in0=sc_a[:, sh:NT],
                                            in1=sc_a[:, 0:NT - sh],
                                            op=ALU.add)
                    sc_a, sc_b = sc_b, sc_a
                offs_ex = gt.tile([1, NT], f32)
                nc.vector.tensor_tensor(out=offs_ex[:], in0=sc_a[:],
                                        in1=tot[:], op=ALU.subtract)

                offs_ps = pps.tile([P, NT], f32)
                nc.tensor.matmul(offs_ps[:], ones1[:], offs_ex[:])
                slot = gt.tile([P, NT], f32)
                nc.vector.tensor_tensor(out=slot[:], in0=pos_in[:],
                                        in1=offs_ps[:], op=ALU.add)

            # slot_s = slot*m + TRASH*(1-m)
            sm = gt.tile([P, NT], f32)
            nc.vector.tensor_tensor(out=sm[:], in0=slot[:], in1=msk[:],
                                    op=ALU.mult)
            minv = gt.tile([P, NT], f32)
            nc.vector.tensor_scalar(minv[:], msk[:], -float(TRASH),
                                    float(TRASH), ALU.mult, ALU.add)
            slot_s = gt.tile([P, NT], f32)
            nc.vector.tensor_tensor(out=slot_s[:], in0=sm[:], in1=minv[:],
                                    op=ALU.add)
            slot_i = gt.tile([P, NT], i32)
            nc.vector.tensor_copy(slot_i[:], slot_s[:])

            pack = gt.tile([P, NT, 8], f32)
            nc.vector.memset(pack[:], 0.0)
            nc.vector.tensor_copy(pack[:, :, 0], iotatok[:])
            nc.vector.tensor_copy(pack[:, :, 1], g_e[:])

            for t in range(NT):
                nc.gpsimd.indirect_dma_start(
                    out=scr_d[:],
                    out_offset=IndirectOffsetOnAxis(ap=slot_i[:, t:t + 1],
                                                    axis=0),
                    in_=pack[:, t, :], in_offset=None)

            # read back compacted (token_id, gate) list
            idxg_f = gt.tile([P, NTC, 2], f32)
            nc.sync.dma_start(
                idxg_f[:],
                scr_d[0:C_CAP, 0:2].rearrange("(t p) c -> p t c", p=P))
            idx_f = gt.tile([P, NTC], f32)
            nc.vector.tensor_copy(idx_f[:], idxg_f[:, :, 0])
            g_all = gt.tile([P, NTC], f32)
            nc.vector.tensor_copy(g_all[:], idxg_f[:, :, 1])
            idx_i = gt.tile([P, NTC], i32)
            nc.vector.tensor_copy(idx_i[:], idx_f[:])

            idx_gather = gt.tile([P, NTC], i32)
            nc.vector.tensor_scalar_max(idx_gather[:], idx_i[:], 0)
            mneg = gt.tile([P, NTC], i32)
            nc.vector.tensor_scalar(mneg[:], idx_i[:], 0, None, ALU.is_lt)
            nc.vector.tensor_scalar(mneg[:], mneg[:], N + 1, None, ALU.mult)
            idx_scat = gt.tile([P, NTC], i32)
            nc.vector.tensor_tensor(out=idx_scat[:], in0=idx_i[:],
                                    in1=mneg[:], op=ALU.add)

            # ================= expert FFN (static slices) =================
            fx = ctx.enter_context(tc.tile_pool(name="fx", bufs=6))
            fxT = ctx.enter_context(tc.tile_pool(name="fxT", bufs=2))
            fh = ctx.enter_context(tc.tile_pool(name="fh", bufs=1))
            fw1 = ctx.enter_context(tc.tile_pool(name="fw1", bufs=3))
            fw2 = ctx.enter_context(tc.tile_pool(name="fw2", bufs=3))
            fyT = ctx.enter_context(tc.tile_pool(name="fyT", bufs=1))
            fy = ctx.enter_context(tc.tile_pool(name="fy", bufs=6))
            ptr = ctx.enter_context(tc.tile_pool(name="ptr", bufs=2,
                                                 space="PSUM"))
            pf1 = ctx.enter_context(tc.tile_pool(name="pf1", bufs=2,
                                                 space="PSUM"))
            pf2 = ctx.enter_context(tc.tile_pool(name="pf2", bufs=1,
                                                 space="PSUM"))

            for tb, cs in SLICES:
                tps = cs // P
                # gather token tiles
                xe = []
                for k in range(tps):
                    xt = fx.tile([P, D], f32, tag="xe", name=f"xe_{tb}_{k}")
                    nc.gpsimd.indirect_dma_start(
                        out=xt[:], out_offset=None, in_=x_d[:],
                        in_offset=IndirectOffsetOnAxis(
                            ap=idx_gather[:, tb + k:tb + k + 1], axis=0))
                    xe.append(xt)

                # transpose to xeT [128, DT, cs] (f32r via the DVE copy)
                xeT = fxT.tile([P, DT, cs], f32r, tag="xeT",
                               name=f"xeT_{tb}")
                for k in range(tps):
                    for dt_i in range(DT):
                        tp = ptr.tile([P, P], f32, tag="tp", name="tp")
                        nc.tensor.transpose(
                            tp[:], xe[k][:, dt_i * P:(dt_i + 1) * P],
                            ident[:])
                        nc.vector.tensor_copy(
                            xeT[:, dt_i, k * P:(k + 1) * P], tp[:])

                # FFN1 + relu+b1 -> hT [128, HT, cs]
                hT = fh.tile([P, HT, cs], f32r, tag="hT", name=f"hT_{tb}")
                for ht in range(HT):
                    w1t = fw1.tile([P, DT, P], f32r, tag="w1t", name="w1t")
                    nc.sync.dma_start(
                        w1t[:],
                        w1_d[:, ht * P:(ht + 1) * P].rearrange(
                            "(a p) c -> p a c", p=P))
                    ps1 = pf1.tile([P, cs], f32, tag="ps1", name="ps1")
                    for dt_i in range(DT):
                        nc.tensor.matmul(ps1[:], w1t[:, dt_i, :],
                                         xeT[:, dt_i, :],
                                         start=(dt_i == 0),
                                         stop=(dt_i == DT - 1))
                    nc.scalar.activation(hT[:, ht, :], ps1[:], AF.Relu,
                                         bias=b1_sb[:, ht:ht + 1])

                # FFN2 (2 groups of 4 d-tiles) -> yT [128, DT, cs]
                yT = fyT.tile([P, DT, cs], f32, tag="yT", name=f"yT_{tb}")
                for dg in range(2):
                    ps2 = [pf2.tile([P, cs], f32, tag=f"ps2_{i}",
                                    name=f"ps2_{tb}_{dg}_{i}")
                           for i in range(4)]
                    for ht in range(HT):
                        w2t = fw2.tile([P, 512], f32r, tag="w2t", name="w2t")
                        nc.sync.dma_start(
                            w2t[:],
                            w2_d[ht * P:(ht + 1) * P,
                                 dg * 512:(dg + 1) * 512])
                        for i in range(4):
                            nc.tensor.matmul(
                                ps2[i][:], w2t[:, i * P:(i + 1) * P],
                                hT[:, ht, :],
                                start=(ht == 0), stop=(ht == HT - 1))
                    for i in range(4):
                        di = dg * 4 + i
                        nc.scalar.activation(yT[:, di, :], ps2[i][:],
                                             AF.Identity,
                                             bias=b2_sb[:, di:di + 1])

                # transpose back, scale by gate, scatter
                for k in range(tps):
                    yk = fy.tile([P, D], f32, tag="yk", name=f"yk_{tb}_{k}")
                    for dt_i in range(DT):
                        tp2 = ptr.tile([P, P], f32, tag="tp", name="tp2")
                        nc.tensor.transpose(
                            tp2[:], yT[:, dt_i, k * P:(k + 1) * P], ident[:])
                        nc.vector.tensor_scalar_mul(
                            yk[:, dt_i * P:(dt_i + 1) * P], tp2[:],
                            g_all[:, tb + k:tb + k + 1])
                    nc.gpsimd.indirect_dma_start(
                        out=out_d[:],
                        out_offset=IndirectOffsetOnAxis(
                            ap=idx_scat[:, tb + k:tb + k + 1], axis=0),
                        in_=yk[:], in_offset=None)

    nc.compile()
    _NC_CACHE[key] = nc
    return nc


def kernel(x, Wg, bg, W1, b1, W2, b2):
    x = np.ascontiguousarray(np.asarray(x, dtype=np.float32))
    Wg = np.ascontiguousarray(np.asarray(Wg, dtype=np.float32))
    bg = np.asarray(bg, dtype=np.float32)
    W1 = np.asarray(W1, dtype=np.float32)
    b1 = np.asarray(b1, dtype=np.float32)
    W2 = np.asarray(W2, dtype=np.float32)
    b2 = np.asarray(b2, dtype=np.float32)

    nc = build()
    xT = np.ascontiguousarray(x.T)
    iota8 = np.tile(np.arange(8, dtype=np.float32)[None, :], (P, 1))
    iota8r = np.tile((7 - np.arange(8, dtype=np.float32))[None, :], (P, 1))
    # token-major convention: tile t, partition p -> token t*128 + p
    iotatok = (np.arange(NT, dtype=np.float32)[None, :] * P
               + np.arange(P, dtype=np.float32)[:, None])
    ident = np.eye(P, dtype=np.float32)
    utri = np.triu(np.ones((P, P), np.float32))     # utri[q, p] = q <= p
    ones1 = np.ones((1, P), np.float32)
    fillrow = np.zeros((P, 8), np.float32)
    fillrow[:, 0] = -1.0
    bgc = np.ascontiguousarray(bg[:, None])

    core_ids = list(range(E))
    in_maps = []
    for e in core_ids:
        in_maps.append({
            "x": x, "xT": xT, "Wg": Wg, "bgc": bgc,
            "W1e": np.ascontiguousarray(W1[e]),
            "b1e": np.ascontiguousarray(b1[e][None, :]),
            "W2e": np.ascontiguousarray(W2[e]),
            "b2e": np.ascontiguousarray(b2[e][None, :]),
            "eid": np.full((P, 1), e, np.float32),
            "iota8": iota8, "iota8r": iota8r, "iotatok": iotatok,
            "ident": ident, "utri": utri, "ones1": ones1,
            "onescol": ones1.T.copy(), "fillrow": fillrow,
        })

    kwargs = {}
    if TRACE:
        kwargs = dict(trace=True, trace_cores=core_ids)
    res = run_bass_kernel_spmd(nc, in_maps, core_ids, **kwargs)
    LAST["res"] = res
    out = np.zeros((N, D), np.float32)
    for e in core_ids:
        out += res.results[e]["ypart"][:N]
    return out


# revision 17
# speedup vs baseline: 1.3247x; 1.3247x over previous
"""Expert-parallel MoE layer for Trainium2 (8 NeuronCores, 1 expert per core).

Fully static kernel (no dynamic control flow, no gpsimd ucode — neither is
available in this runtime):

Per core e:
  gating:   logitsT = Wg.T @ xT (fp32 matmuls, tie-exact) + bg; softmax +
            top-2 with DVE/ACT ops
  routing:  mask for expert e -> exclusive prefix sum (PE matmul with a
            triangular-ones matrix) -> compacted slot per token ->
            indirect-DMA scatter of (token_id, gate) rows into DRAM
            scratch -> read back the compacted list
  ffn:      fixed 1280-token capacity (counts ~1024±28), H processed in 4
            quarters so W1/W2 stream exactly once; FFN2 accumulates in an
            SBUF tensor across quarters. f32r matmuls, moving dim
            512/512/256 chunks. Gather/scatter by indirect DMA; partial
            output [N+1, D] (row N = trash row for padding).
Host: shards W1/b1/W2/b2 by expert (pre-tiled so every SBUF tile is one
contiguous DMA), replicates x/Wg, sums the 8 partials.

Hardcoded for: N=4096 tokens, D=1024, H=4096, E=8 experts, top-k=2.
"""

import numpy as np
from contextlib import ExitStack

import concourse.bass as bass
import concourse.mybir as mybir
import concourse.tile as tile
from concourse import bacc
from concourse.bass import ds, IndirectOffsetOnAxis
from concourse.bass_utils import run_bass_kernel_spmd

N, D, H, E, TOPK = 4096, 1024, 4096, 8, 2
P = 128
NT = N // P                       # 32 token tiles
DT = D // P                       # 8 d-tiles
HT = H // P                       # 32 h-tiles
NQ = 4                            # h quarters
HTQ = HT // NQ                    # 8 h-tiles per quarter
C_CAP = 1280                      # static routed-token capacity per expert
NTC = C_CAP // P                  # 10 capacity tiles
CHUNKS = [(0, 512), (512, 512), (1024, 256)]   # (col offset, width)
SCR_ROWS = 4224                   # compaction scratch rows; last = trash
TRASH = SCR_ROWS - 1

f32 = mybir.dt.float32
f32r = mybir.dt.float32r
i32 = mybir.dt.int32
AF = mybir.ActivationFunctionType
ALU = mybir.AluOpType

_NC_CACHE = {}

TRACE = False
LAST = {}


def build():
    key = "moe_static_v3"
    if key in _NC_CACHE:
        return _NC_CACHE[key]

    nc = bacc.Bacc("TRN2", target_bir_lowering=False, debug=False)

    x_d = nc.declare_dram_parameter("x", [N, D], f32, isOutput=False)
    # xTt[dt, s, p, c] = x[s*512+c, dt*128+p]  (gating, fp32)
    xtt_d = nc.declare_dram_parameter("xTt", [DT, 8, P, 512], f32,
                                      isOutput=False)
    wg_d = nc.declare_dram_parameter("Wg", [D, E], f32, isOutput=False)
    bgc_d = nc.declare_dram_parameter("bgc", [E, 1], f32, isOutput=False)
    # W1t[ht, p, dt, c] = W1e[dt*128+p, ht*128+c]
    w1_d = nc.declare_dram_parameter("W1t", [HT, P, DT, P], f32r,
                                     isOutput=False)
    b1_d = nc.declare_dram_parameter("b1e", [1, H], f32, isOutput=False)
    # W2t[ht, d, p, c] = W2e[ht*128+p, d*128+c]
    w2_d = nc.declare_dram_parameter("W2t", [HT, DT, P, P], f32r,
                                     isOutput=False)
    b2_d = nc.declare_dram_parameter("b2e", [1, D], f32, isOutput=False)
    eid_d = nc.declare_dram_parameter("eid", [P, 1], f32, isOutput=False)
    iota_d = nc.declare_dram_parameter("iota8", [P, 8], f32, isOutput=False)
    iotar_d = nc.declare_dram_parameter("iota8r", [P, 8], f32, isOutput=False)
    itok_d = nc.declare_dram_parameter("iotatok", [P, NT], f32, isOutput=False)
    ident_d = nc.declare_dram_parameter("ident", [P, P], f32, isOutput=False)
    ut_d = nc.declare_dram_parameter("utri", [P, P], f32, isOutput=False)
    ones1_d = nc.declare_dram_parameter("ones1", [1, P], f32, isOutput=False)
    onesc_d = nc.declare_dram_parameter("onescol", [P, 1], f32, isOutput=False)
    fill_d = nc.declare_dram_parameter("fillrow", [P, 8], f32, isOutput=False)

    out_d = nc.declare_dram_parameter("ypart", [N + 1, D], f32, isOutput=True)
    scr_d = nc.dram_tensor("scratch", [SCR_ROWS, 8], f32)

    with tile.TileContext(nc) as tc:
        with ExitStack() as ctx:
            cpool = ctx.enter_context(tc.tile_pool(name="consts", bufs=1))

            ident = cpool.tile([P, P], f32)
            nc.sync.dma_start(ident[:], ident_d[:])
            utri = cpool.tile([P, P], f32)
            nc.sync.dma_start(utri[:], ut_d[:])
            ones1 = cpool.tile([1, P], f32)
            nc.sync.dma_start(ones1[:], ones1_d[:])
            onescol = cpool.tile([P, 1], f32)
            nc.sync.dma_start(onescol[:], onesc_d[:])
            iota8 = cpool.tile([P, 8], f32)
            nc.sync.dma_start(iota8[:], iota_d[:])
            iota8r = cpool.tile([P, 8], f32)
            nc.sync.dma_start(iota8r[:], iotar_d[:])
            iotatok = cpool.tile([P, NT], f32)
            nc.sync.dma_start(iotatok[:], itok_d[:])
            eid = cpool.tile([P, 1], f32)
            nc.sync.dma_start(eid[:], eid_d[:])
            fillrow = cpool.tile([P, 8], f32)
            nc.sync.dma_start(fillrow[:], fill_d[:])
            bgc = cpool.tile([E, 1], f32)
            nc.sync.dma_start(bgc[:], bgc_d[:])
            wg_sb = cpool.tile([P, DT, E], f32)
            for dt_i in range(DT):
                nc.sync.dma_start(wg_sb[:, dt_i, :],
                                  wg_d[dt_i * P:(dt_i + 1) * P, :])
            b1_sb = cpool.tile([P, HT], f32)
            nc.sync.dma_start(b1_sb[:], b1_d[0, :].rearrange("(a p) -> p a", p=P))
            b2_sb = cpool.tile([P, DT], f32)
            nc.sync.dma_start(b2_sb[:], b2_d[0, :].rearrange("(a p) -> p a", p=P))

            zero_t = cpool.tile([P, D], f32)
            nc.vector.memset(zero_t[:], 0.0)

            # zero-fill partial output; prefill compaction scratch
            for blk in range(N // P):
                nc.sync.dma_start(out_d[blk * P:(blk + 1) * P, :], zero_t[:])
            nc.sync.dma_start(out_d[N:N + 1, :], zero_t[0:1, :])
            for t in range(NTC):
                nc.sync.dma_start(scr_d[t * P:(t + 1) * P, :], fillrow[:])

            # ================= gating =================
            logits = cpool.tile([P, NT, 8], f32)
            with ExitStack() as gctx:
                gx = gctx.enter_context(tc.tile_pool(name="gx", bufs=3))
                gps = gctx.enter_context(
                    tc.tile_pool(name="gps", bufs=2, space="PSUM"))
                gtp = gctx.enter_context(
                    tc.tile_pool(name="gtp", bufs=2, space="PSUM"))
                gsb = gctx.enter_context(tc.tile_pool(name="gsb", bufs=2))

                GCS = 512
                for s in range(N // GCS):
                    lg_ps = gps.tile([E, GCS], f32, tag="lg")
                    for dt_i in range(DT):
                        xt_t = gx.tile([P, GCS], f32, tag="xt")
                        nc.sync.dma_start(xt_t[:], xtt_d[dt_i, s])
                        nc.tensor.matmul(
                            lg_ps[:], wg_sb[:, dt_i, :], xt_t[:],
                            start=(dt_i == 0), stop=(dt_i == DT - 1))
                    lg_sb = gsb.tile([E, GCS], f32, tag="lgsb")
                    nc.scalar.activation(lg_sb[:], lg_ps[:], AF.Identity,
                                         bias=bgc[:])
                    for c in range(GCS // P):
                        tr_ps = gtp.tile([P, 8], f32, tag="tr")
                        nc.tensor.transpose(tr_ps[:],
                                            lg_sb[:, c * P:(c + 1) * P],
                                            ident[0:E, 0:E])
                        nc.vector.tensor_copy(logits[:, s * (GCS // P) + c, :],
                                              tr_ps[:])

            # softmax + top2 (token-major [128, NT, 8])
            gt = ctx.enter_context(tc.tile_pool(name="gt", bufs=1))
            negm = gt.tile([P, NT], f32)
            nc.vector.tensor_reduce(negm[:], logits[:], mybir.AxisListType.X,
                                    ALU.max, negate=True)
            e_sb = gt.tile([P, NT, 8], f32)
            for t in range(NT):
                nc.scalar.activation(e_sb[:, t, :], logits[:, t, :], AF.Exp,
                                     bias=negm[:, t:t + 1])
            ssum = gt.tile([P, NT], f32)
            nc.vector.tensor_reduce(ssum[:], e_sb[:], mybir.AxisListType.X,
                                    ALU.add)
            rsum = gt.tile([P, NT], f32)
            nc.vector.reciprocal(rsum[:], ssum[:])

            def argmax8(e_t, tag):
                mx = gt.tile([P, NT], f32, tag=tag + "mx", name=tag + "mx")
                nc.vector.tensor_reduce(mx[:], e_t[:], mybir.AxisListType.X,
                                        ALU.max)
                eq = gt.tile([P, NT, 8], f32, tag=tag + "eq", name=tag + "eq")
                nc.vector.tensor_tensor(
                    out=eq[:], in0=e_t[:],
                    in1=mx[:, :, None].to_broadcast([P, NT, 8]),
                    op=ALU.is_ge)
                rv = gt.tile([P, NT, 8], f32, tag=tag + "rv", name=tag + "rv")
                nc.vector.tensor_tensor(
                    out=rv[:], in0=eq[:],
                    in1=iota8r[:, None, :].to_broadcast([P, NT, 8]),
                    op=ALU.mult)
                im = gt.tile([P, NT], f32, tag=tag + "im", name=tag + "im")
                nc.vector.tensor_reduce(im[:], rv[:], mybir.AxisListType.X,
                                        ALU.max)
                idx = gt.tile([P, NT], f32, tag=tag + "idx", name=tag + "idx")
                nc.vector.tensor_scalar(idx[:], im[:], -1.0, 7.0,
                                        ALU.mult, ALU.add)
                return mx, idx

            e1, idx1 = argmax8(e_sb, "a1")
            mask1 = gt.tile([P, NT, 8], f32)
            nc.vector.tensor_tensor(
                out=mask1[:],
                in0=iota8[:, None, :].to_broadcast([P, NT, 8]),
                in1=idx1[:, :, None].to_broadcast([P, NT, 8]),
                op=ALU.is_equal)
            e_m = gt.tile([P, NT, 8], f32)
            nc.vector.tensor_tensor(out=e_m[:], in0=e_sb[:], in1=mask1[:],
                                    op=ALU.mult)
            e_rest = gt.tile([P, NT, 8], f32)
            nc.vector.tensor_tensor(out=e_rest[:], in0=e_sb[:], in1=e_m[:],
                                    op=ALU.subtract)
            e2, idx2 = argmax8(e_rest, "a2")

            # this expert's mask + gate value per token
            eq1 = gt.tile([P, NT], f32)
            nc.vector.tensor_scalar(eq1[:], idx1[:], eid[:, 0:1], None,
                                    ALU.is_equal)
            eq2 = gt.tile([P, NT], f32)
            nc.vector.tensor_scalar(eq2[:], idx2[:], eid[:, 0:1], None,
                                    ALU.is_equal)
            msk = gt.tile([P, NT], f32)
            nc.vector.tensor_tensor(out=msk[:], in0=eq1[:], in1=eq2[:],
                                    op=ALU.add)
            ge1 = gt.tile([P, NT], f32)
            nc.vector.tensor_tensor(out=ge1[:], in0=e1[:], in1=eq1[:],
                                    op=ALU.mult)
            ge2 = gt.tile([P, NT], f32)
            nc.vector.tensor_tensor(out=ge2[:], in0=e2[:], in1=eq2[:],
                                    op=ALU.mult)
            gsum = gt.tile([P, NT], f32)
            nc.vector.tensor_tensor(out=gsum[:], in0=ge1[:], in1=ge2[:],
                                    op=ALU.add)
            g_e = gt.tile([P, NT], f32)
            nc.vector.tensor_tensor(out=g_e[:], in0=gsum[:], in1=rsum[:],
                                    op=ALU.mult)

            # ============ compaction (prefix sum + scatter) ============
            with ExitStack() as pctx:
                pps = pctx.enter_context(
                    tc.tile_pool(name="pps", bufs=1, space="PSUM"))
                cum_ps = pps.tile([P, NT], f32)
                nc.tensor.matmul(cum_ps[:], utri[:], msk[:])
                cum = gt.tile([P, NT], f32)
                nc.vector.tensor_copy(cum[:], cum_ps[:])
                pos_in = gt.tile([P, NT], f32)
                nc.vector.tensor_tensor(out=pos_in[:], in0=cum[:], in1=msk[:],
                                        op=ALU.subtract)

                tot_ps = pps.tile([1, NT], f32, name="tot_ps")
                nc.tensor.matmul(tot_ps[:], onescol[:], msk[:])
                tot = gt.tile([1, NT], f32)
                nc.vector.tensor_copy(tot[:], tot_ps[:])
                sc_a = gt.tile([1, NT], f32)
                nc.vector.tensor_copy(sc_a[:], tot[:])
                sc_b = gt.tile([1, NT], f32)
                for sh in (1, 2, 4, 8, 16):
                    nc.vector.tensor_copy(sc_b[:, 0:sh], sc_a[:, 0:sh])
                    nc.vector.tensor_tensor(out=sc_b[:, sh:NT],
                                            in0=sc_a[:, sh:NT],
                                            in1=sc_a[:, 0:NT - sh],
                                            op=ALU.add)
                    sc_a, sc_b = sc_b, sc_a
                offs_ex = gt.tile([1, NT], f32)
                nc.vector.tensor_tensor(out=offs_ex[:], in0=sc_a[:],
                                        in1=tot[:], op=ALU.subtract)

                offs_ps = pps.tile([P, NT], f32)
                nc.tensor.matmul(offs_ps[:], ones1[:], offs_ex[:])
                slot = gt.tile([P, NT], f32)
                nc.vector.tensor_tensor(out=slot[:], in0=pos_in[:],
                                        in1=offs_ps[:], op=ALU.add)

            # slot_s = slot*m + TRASH*(1-m)
            sm = gt.tile([P, NT], f32)
            nc.vector.tensor_tensor(out=sm[:], in0=slot[:], in1=msk[:],
                                    op=ALU.mult)
            minv = gt.tile([P, NT], f32)
            nc.vector.tensor_scalar(minv[:], msk[:], -float(TRASH),
                                    float(TRASH), ALU.mult, ALU.add)
            slot_s = gt.tile([P, NT], f32)
            nc.vector.tensor_tensor(out=slot_s[:], in0=sm[:], in1=minv[:],
                                    op=ALU.add)
            slot_i = gt.tile([P, NT], i32)
            nc.vector.tensor_copy(slot_i[:], slot_s[:])

            pack = gt.tile([P, NT, 8], f32)
            nc.vector.memset(pack[:], 0.0)
            nc.vector.tensor_copy(pack[:, :, 0], iotatok[:])
            nc.vector.tensor_copy(pack[:, :, 1], g_e[:])

            for t in range(NT):
                nc.gpsimd.indirect_dma_start(
                    out=scr_d[:],
                    out_offset=IndirectOffsetOnAxis(ap=slot_i[:, t:t + 1],
                                                    axis=0),
                    in_=pack[:, t, :], in_offset=None)

            # read back compacted (token_id, gate) list
            idxg_f = gt.tile([P, NTC, 2], f32)
            nc.sync.dma_start(
                idxg_f[:],
                scr_d[0:C_CAP, 0:2].rearrange("(t p) c -> p t c", p=P))
            idx_f = gt.tile([P, NTC], f32)
            nc.vector.tensor_copy(idx_f[:], idxg_f[:, :, 0])
            g_all = gt.tile([P, NTC], f32)
            nc.vector.tensor_copy(g_all[:], idxg_f[:, :, 1])
            idx_i = gt.tile([P, NTC], i32)
            nc.vector.tensor_copy(idx_i[:], idx_f[:])

            idx_gather = gt.tile([P, NTC], i32)
            nc.vector.tensor_scalar_max(idx_gather[:], idx_i[:], 0)
            mneg = gt.tile([P, NTC], i32)
            nc.vector.tensor_scalar(mneg[:], idx_i[:], 0, None, ALU.is_lt)
            nc.vector.tensor_scalar(mneg[:], mneg[:], N + 1, None, ALU.mult)
            idx_scat = gt.tile([P, NTC], i32)
            nc.vector.tensor_tensor(out=idx_scat[:], in0=idx_i[:],
                                    in1=mneg[:], op=ALU.add)

            # ================= expert FFN =================
            fx = ctx.enter_context(tc.tile_pool(name="fx", bufs=4))
            fbig = ctx.enter_context(tc.tile_pool(name="fbig", bufs=1))
            fw1 = ctx.enter_context(tc.tile_pool(name="fw1", bufs=3))
            fw2 = ctx.enter_context(tc.tile_pool(name="fw2", bufs=3))
            fy = ctx.enter_context(tc.tile_pool(name="fy", bufs=3))
            ptr = ctx.enter_context(tc.tile_pool(name="ptr", bufs=2,
                                                 space="PSUM"))
            pf1 = ctx.enter_context(tc.tile_pool(name="pf1", bufs=2,
                                                 space="PSUM"))
            pf2 = ctx.enter_context(tc.tile_pool(name="pf2", bufs=1,
                                                 space="PSUM"))

            # gather all capacity tiles, transpose to xeT [128, DT, C_CAP]
            xeT = fbig.tile([P, DT, C_CAP], f32r, name="xeT")
            for k in range(NTC):
                xt = fx.tile([P, D], f32, tag="xe", name=f"xe_{k}")
                nc.gpsimd.indirect_dma_start(
                    out=xt[:], out_offset=None, in_=x_d[:],
                    in_offset=IndirectOffsetOnAxis(
                        ap=idx_gather[:, k:k + 1], axis=0))
                for dt_i in range(DT):
                    tp = ptr.tile([P, P], f32, tag="tp", name="tp")
                    nc.tensor.transpose(
                        tp[:], xt[:, dt_i * P:(dt_i + 1) * P], ident[:])
                    nc.vector.tensor_copy(
                        xeT[:, dt_i, k * P:(k + 1) * P], tp[:])

            hT = fbig.tile([P, HTQ, C_CAP], f32r, name="hT")
            yT = fbig.tile([P, DT, C_CAP], f32, name="yT")

            for q in range(NQ):
                # FFN1 for this h-quarter
                for ht_l in range(HTQ):
                    ht = q * HTQ + ht_l
                    w1t = fw1.tile([P, DT, P], f32r, tag="w1t", name="w1t")
                    nc.sync.dma_start(w1t[:], w1_d[ht])
                    for co, cw in CHUNKS:
                        ps1 = pf1.tile([P, 512], f32, tag="ps1", name="ps1")
                        for dt_i in range(DT):
                            nc.tensor.matmul(ps1[:, 0:cw],
                                             w1t[:, dt_i, :],
                                             xeT[:, dt_i, co:co + cw],
                                             start=(dt_i == 0),
                                             stop=(dt_i == DT - 1))
                        nc.scalar.activation(hT[:, ht_l, co:co + cw],
                                             ps1[:, 0:cw], AF.Relu,
                                             bias=b1_sb[:, ht:ht + 1])

                # FFN2: one d-tile at a time, accumulate into yT (SBUF)
                for d_i in range(DT):
                    ps2 = [pf2.tile([P, cw], f32, tag=f"ps2_{ci}",
                                    name=f"ps2_{q}_{d_i}_{ci}")
                           for ci, (co, cw) in enumerate(CHUNKS)]
                    for ht_l in range(HTQ):
                        ht = q * HTQ + ht_l
                        w2t = fw2.tile([P, P], f32r, tag="w2t", name="w2t")
                        nc.sync.dma_start(w2t[:], w2_d[ht, d_i])
                        for ci, (co, cw) in enumerate(CHUNKS):
                            nc.tensor.matmul(ps2[ci][:], w2t[:],
                                             hT[:, ht_l, co:co + cw],
                                             start=(ht_l == 0),
                                             stop=(ht_l == HTQ - 1))
                    for ci, (co, cw) in enumerate(CHUNKS):
                        if q == 0:
                            nc.scalar.activation(yT[:, d_i, co:co + cw],
                                                 ps2[ci][:], AF.Identity,
                                                 bias=b2_sb[:, d_i:d_i + 1])
                        else:
                            nc.vector.tensor_tensor(
                                out=yT[:, d_i, co:co + cw],
                                in0=ps2[ci][:],
                                in1=yT[:, d_i, co:co + cw],
                                op=ALU.add)

            # transpose back, scale by gate, scatter
            for k in range(NTC):
                yk = fy.tile([P, D], f32, tag="yk", name=f"yk_{k}")
                for dt_i in range(DT):
                    tp2 = ptr.tile([P, P], f32, tag="tp", name="tp2")
                    nc.tensor.transpose(
                        tp2[:], yT[:, dt_i, k * P:(k + 1) * P], ident[:])
                    nc.vector.tensor_scalar_mul(
                        yk[:, dt_i * P:(dt_i + 1) * P], tp2[:],
                        g_all[:, k:k + 1])
                nc.gpsimd.indirect_dma_start(
                    out=out_d[:],
                    out_offset=IndirectOffsetOnAxis(
                        ap=idx_scat[:, k:k + 1], axis=0),
                    in_=yk[:], in_offset=None)

    nc.compile()
    _NC_CACHE[key] = nc
    return nc


def make_feeds(x, Wg, bg, W1, b1, W2, b2, e):
    """Per-core input map (host-side sharding + layout prep)."""
    xTt = np.ascontiguousarray(
        x.reshape(8, 512, DT, P).transpose(2, 0, 3, 1))
    w1t = np.ascontiguousarray(
        W1[e].reshape(DT, P, HT, P).transpose(2, 1, 0, 3))
    w2t = np.ascontiguousarray(
        W2[e].reshape(HT, P, DT, P).transpose(0, 2, 1, 3))
    iota8 = np.tile(np.arange(8, dtype=np.float32)[None, :], (P, 1))
    iota8r = np.tile((7 - np.arange(8, dtype=np.float32))[None, :], (P, 1))
    iotatok = (np.arange(NT, dtype=np.float32)[None, :] * P
               + np.arange(P, dtype=np.float32)[:, None])
    fillrow = np.zeros((P, 8), np.float32)
    fillrow[:, 0] = -1.0
    return {
        "x": x, "xTt": xTt, "Wg": Wg,
        "bgc": np.ascontiguousarray(bg[:, None]),
        "W1t": w1t, "b1e": np.ascontiguousarray(b1[e][None, :]),
        "W2t": w2t, "b2e": np.ascontiguousarray(b2[e][None, :]),
        "eid": np.full((P, 1), e, np.float32),
        "iota8": iota8, "iota8r": iota8r, "iotatok": iotatok,
        "ident": np.eye(P, dtype=np.float32),
        "utri": np.triu(np.ones((P, P), np.float32)),
        "ones1": np.ones((1, P), np.float32),
        "onescol": np.ones((P, 1), np.float32),
        "fillrow": fillrow,
    }


def kernel(x, Wg, bg, W1, b1, W2, b2):
    x = np.ascontiguousarray(np.asarray(x, dtype=np.float32))
    Wg = np.ascontiguousarray(np.asarray(Wg, dtype=np.float32))
    bg = np.asarray(bg, dtype=np.float32)
    W1 = np.asarray(W1, dtype=np.float32)
    b1 = np.asarray(b1, dtype=np.float32)
    W2 = np.asarray(W2, dtype=np.float32)
    b2 = np.asarray(b2, dtype=np.float32)

    nc = build()
    core_ids = list(range(E))
    in_maps = [make_feeds(x, Wg, bg, W1, b1, W2, b2, e) for e in core_ids]

    kwargs = {}
    if TRACE:
        kwargs = dict(trace=True, trace_cores=core_ids)
    res = run_bass_kernel_spmd(nc, in_maps, core_ids, **kwargs)
    LAST["res"] = res
    out = np.zeros((N, D), np.float32)
    for e in core_ids:
        out += res.results[e]["ypart"][:N]
    return out


# revision 20
# speedup vs baseline: 1.5579x; 1.1760x over previous
"""Expert-parallel MoE layer for Trainium2 (8 NeuronCores, 1 expert per core).

Fully static kernel (no dynamic control flow, no gpsimd ucode — neither is
available in this runtime):

Per core e:
  gating:   logitsT = Wg.T @ xT (fp32 matmuls, tie-exact) + bg; softmax +
            top-2 with DVE/ACT ops
  routing:  mask for expert e -> exclusive prefix sum (PE matmul with a
            triangular-ones matrix) -> compacted slot per token ->
            indirect-DMA scatter of (token_id, gate) rows into DRAM
            scratch -> read back the compacted list
  ffn:      fixed 1280-token capacity (counts ~1024±28), H processed in 4
            quarters so W1/W2 stream exactly once; FFN2 accumulates in an
            SBUF tensor across quarters. f32r matmuls, moving dim
            512/512/256 chunks. Gather/scatter by indirect DMA; partial
            output [N+1, D] (row N = trash row for padding).
Host: shards W1/b1/W2/b2 by expert (pre-tiled so every SBUF tile is one
contiguous DMA), replicates x/Wg, sums the 8 partials.

Hardcoded for: N=4096 tokens, D=1024, H=4096, E=8 experts, top-k=2.
"""

import numpy as np
from contextlib import ExitStack

import concourse.bass as bass
import concourse.mybir as mybir
import concourse.tile as tile
from concourse import bacc
from concourse.bass import ds, IndirectOffsetOnAxis
from concourse.bass_utils import run_bass_kernel_spmd

N, D, H, E, TOPK = 4096, 1024, 4096, 8, 2
P = 128
NT = N // P                       # 32 token tiles
DT = D // P                       # 8 d-tiles
HT = H // P                       # 32 h-tiles
NQ = 4                            # h quarters
HTQ = HT // NQ                    # 8 h-tiles per quarter
C_CAP = 1280                      # static routed-token capacity per expert
NTC = C_CAP // P                  # 10 capacity tiles
CHUNKS = [(0, 512), (512, 512), (1024, 256)]   # (col offset, width)
SCR_ROWS = 4224                   # compaction scratch rows; last = trash
TRASH = SCR_ROWS - 1

f32 = mybir.dt.float32
f32r = mybir.dt.float32r
i32 = mybir.dt.int32
AF = mybir.ActivationFunctionType
ALU = mybir.AluOpType

_NC_CACHE = {}

TRACE = False
LAST = {}


def build():
    key = "moe_static_v3"
    if key in _NC_CACHE:
        return _NC_CACHE[key]

    nc = bacc.Bacc("TRN2", target_bir_lowering=False, debug=False)

    x_d = nc.declare_dram_parameter("x", [N, D], f32, isOutput=False)
    # xTt[dt, s, p, c] = x[s*512+c, dt*128+p]  (gating, fp32)
    xtt_d = nc.declare_dram_parameter("xTt", [DT, 8, P, 512], f32,
                                      isOutput=False)
    wg_d = nc.declare_dram_parameter("Wg", [D, E], f32, isOutput=False)
    bgc_d = nc.declare_dram_parameter("bgc", [E, 1], f32, isOutput=False)
    # W1t[ht, p, dt, c] = W1e[dt*128+p, ht*128+c]
    w1_d = nc.declare_dram_parameter("W1t", [HT, P, DT, P], f32r,
                                     isOutput=False)
    b1_d = nc.declare_dram_parameter("b1e", [1, H], f32, isOutput=False)
    # W2t[q, d, p, ht_l, c] = W2e[(q*HTQ+ht_l)*128+p, d*128+c]
    # (per (q, d) slice: each partition's HTQ*128 elements are contiguous)
    w2_d = nc.declare_dram_parameter("W2t", [NQ, DT, P, HTQ, P], f32r,
                                     isOutput=False)
    b2_d = nc.declare_dram_parameter("b2e", [1, D], f32, isOutput=False)
    eid_d = nc.declare_dram_parameter("eid", [P, 1], f32, isOutput=False)
    iota_d = nc.declare_dram_parameter("iota8", [P, 8], f32, isOutput=False)
    iotar_d = nc.declare_dram_parameter("iota8r", [P, 8], f32, isOutput=False)
    itok_d = nc.declare_dram_parameter("iotatok", [P, NT], f32, isOutput=False)
    ident_d = nc.declare_dram_parameter("ident", [P, P], f32, isOutput=False)
    ut_d = nc.declare_dram_parameter("utri", [P, P], f32, isOutput=False)
    ones1_d = nc.declare_dram_parameter("ones1", [1, P], f32, isOutput=False)
    onesc_d = nc.declare_dram_parameter("onescol", [P, 1], f32, isOutput=False)
    fill_d = nc.declare_dram_parameter("fillrow", [P, 8], f32, isOutput=False)

    out_d = nc.declare_dram_parameter("ypart", [N + 1, D], f32, isOutput=True)
    scr_d = nc.dram_tensor("scratch", [SCR_ROWS, 8], f32)

    with tile.TileContext(nc) as tc:
        with ExitStack() as ctx:
            cpool = ctx.enter_context(tc.tile_pool(name="consts", bufs=1))

            ident = cpool.tile([P, P], f32)
            nc.sync.dma_start(ident[:], ident_d[:])
            utri = cpool.tile([P, P], f32)
            nc.sync.dma_start(utri[:], ut_d[:])
            ones1 = cpool.tile([1, P], f32)
            nc.sync.dma_start(ones1[:], ones1_d[:])
            onescol = cpool.tile([P, 1], f32)
            nc.sync.dma_start(onescol[:], onesc_d[:])
            iota8 = cpool.tile([P, 8], f32)
            nc.sync.dma_start(iota8[:], iota_d[:])
            iota8r = cpool.tile([P, 8], f32)
            nc.sync.dma_start(iota8r[:], iotar_d[:])
            iotatok = cpool.tile([P, NT], f32)
            nc.sync.dma_start(iotatok[:], itok_d[:])
            eid = cpool.tile([P, 1], f32)
            nc.sync.dma_start(eid[:], eid_d[:])
            fillrow = cpool.tile([P, 8], f32)
            nc.sync.dma_start(fillrow[:], fill_d[:])
            bgc = cpool.tile([E, 1], f32)
            nc.sync.dma_start(bgc[:], bgc_d[:])
            wg_sb = cpool.tile([P, DT, E], f32)
            for dt_i in range(DT):
                nc.sync.dma_start(wg_sb[:, dt_i, :],
                                  wg_d[dt_i * P:(dt_i + 1) * P, :])
            b1_sb = cpool.tile([P, HT], f32)
            nc.sync.dma_start(b1_sb[:], b1_d[0, :].rearrange("(a p) -> p a", p=P))
            b2_sb = cpool.tile([P, DT], f32)
            nc.sync.dma_start(b2_sb[:], b2_d[0, :].rearrange("(a p) -> p a", p=P))

            zero_t = cpool.tile([P, D], f32)
            nc.vector.memset(zero_t[:], 0.0)

            # zero-fill partial output; prefill compaction scratch
            for blk in range(N // P):
                nc.sync.dma_start(out_d[blk * P:(blk + 1) * P, :], zero_t[:])
            nc.sync.dma_start(out_d[N:N + 1, :], zero_t[0:1, :])
            for t in range(NTC):
                nc.sync.dma_start(scr_d[t * P:(t + 1) * P, :], fillrow[:])

            # ================= gating =================
            logits = cpool.tile([P, NT, 8], f32)
            with ExitStack() as gctx:
                gx = gctx.enter_context(tc.tile_pool(name="gx", bufs=3))
                gps = gctx.enter_context(
                    tc.tile_pool(name="gps", bufs=2, space="PSUM"))
                gtp = gctx.enter_context(
                    tc.tile_pool(name="gtp", bufs=2, space="PSUM"))
                gsb = gctx.enter_context(tc.tile_pool(name="gsb", bufs=2))

                GCS = 512
                for s in range(N // GCS):
                    lg_ps = gps.tile([E, GCS], f32, tag="lg")
                    for dt_i in range(DT):
                        xt_t = gx.tile([P, GCS], f32, tag="xt")
                        nc.sync.dma_start(xt_t[:], xtt_d[dt_i, s])
                        nc.tensor.matmul(
                            lg_ps[:], wg_sb[:, dt_i, :], xt_t[:],
                            start=(dt_i == 0), stop=(dt_i == DT - 1))
                    lg_sb = gsb.tile([E, GCS], f32, tag="lgsb")
                    nc.scalar.activation(lg_sb[:], lg_ps[:], AF.Identity,
                                         bias=bgc[:])
                    for c in range(GCS // P):
                        tr_ps = gtp.tile([P, 8], f32, tag="tr")
                        nc.tensor.transpose(tr_ps[:],
                                            lg_sb[:, c * P:(c + 1) * P],
                                            ident[0:E, 0:E])
                        nc.vector.tensor_copy(logits[:, s * (GCS // P) + c, :],
                                              tr_ps[:])

            # softmax + top2 (token-major [128, NT, 8])
            gt = ctx.enter_context(tc.tile_pool(name="gt", bufs=1))
            negm = gt.tile([P, NT], f32)
            nc.vector.tensor_reduce(negm[:], logits[:], mybir.AxisListType.X,
                                    ALU.max, negate=True)
            e_sb = gt.tile([P, NT, 8], f32)
            for t in range(NT):
                nc.scalar.activation(e_sb[:, t, :], logits[:, t, :], AF.Exp,
                                     bias=negm[:, t:t + 1])
            ssum = gt.tile([P, NT], f32)
            nc.vector.tensor_reduce(ssum[:], e_sb[:], mybir.AxisListType.X,
                                    ALU.add)
            rsum = gt.tile([P, NT], f32)
            nc.vector.reciprocal(rsum[:], ssum[:])

            def argmax8(e_t, tag):
                mx = gt.tile([P, NT], f32, tag=tag + "mx", name=tag + "mx")
                nc.vector.tensor_reduce(mx[:], e_t[:], mybir.AxisListType.X,
                                        ALU.max)
                eq = gt.tile([P, NT, 8], f32, tag=tag + "eq", name=tag + "eq")
                nc.vector.tensor_tensor(
                    out=eq[:], in0=e_t[:],
                    in1=mx[:, :, None].to_broadcast([P, NT, 8]),
                    op=ALU.is_ge)
                rv = gt.tile([P, NT, 8], f32, tag=tag + "rv", name=tag + "rv")
                nc.vector.tensor_tensor(
                    out=rv[:], in0=eq[:],
                    in1=iota8r[:, None, :].to_broadcast([P, NT, 8]),
                    op=ALU.mult)
                im = gt.tile([P, NT], f32, tag=tag + "im", name=tag + "im")
                nc.vector.tensor_reduce(im[:], rv[:], mybir.AxisListType.X,
                                        ALU.max)
                idx = gt.tile([P, NT], f32, tag=tag + "idx", name=tag + "idx")
                nc.vector.tensor_scalar(idx[:], im[:], -1.0, 7.0,
                                        ALU.mult, ALU.add)
                return mx, idx

            e1, idx1 = argmax8(e_sb, "a1")
            mask1 = gt.tile([P, NT, 8], f32)
            nc.vector.tensor_tensor(
                out=mask1[:],
                in0=iota8[:, None, :].to_broadcast([P, NT, 8]),
                in1=idx1[:, :, None].to_broadcast([P, NT, 8]),
                op=ALU.is_equal)
            e_m = gt.tile([P, NT, 8], f32)
            nc.vector.tensor_tensor(out=e_m[:], in0=e_sb[:], in1=mask1[:],
                                    op=ALU.mult)
            e_rest = gt.tile([P, NT, 8], f32)
            nc.vector.tensor_tensor(out=e_rest[:], in0=e_sb[:], in1=e_m[:],
                                    op=ALU.subtract)
            e2, idx2 = argmax8(e_rest, "a2")

            # this expert's mask + gate value per token
            eq1 = gt.tile([P, NT], f32)
            nc.vector.tensor_scalar(eq1[:], idx1[:], eid[:, 0:1], None,
                                    ALU.is_equal)
            eq2 = gt.tile([P, NT], f32)
            nc.vector.tensor_scalar(eq2[:], idx2[:], eid[:, 0:1], None,
                                    ALU.is_equal)
            msk = gt.tile([P, NT], f32)
            nc.vector.tensor_tensor(out=msk[:], in0=eq1[:], in1=eq2[:],
                                    op=ALU.add)
            ge1 = gt.tile([P, NT], f32)
            nc.vector.tensor_tensor(out=ge1[:], in0=e1[:], in1=eq1[:],
                                    op=ALU.mult)
            ge2 = gt.tile([P, NT], f32)
            nc.vector.tensor_tensor(out=ge2[:], in0=e2[:], in1=eq2[:],
                                    op=ALU.mult)
            gsum = gt.tile([P, NT], f32)
            nc.vector.tensor_tensor(out=gsum[:], in0=ge1[:], in1=ge2[:],
                                    op=ALU.add)
            g_e = gt.tile([P, NT], f32)
            nc.vector.tensor_tensor(out=g_e[:], in0=gsum[:], in1=rsum[:],
                                    op=ALU.mult)

            # ============ compaction (prefix sum + scatter) ============
            with ExitStack() as pctx:
                pps = pctx.enter_context(
                    tc.tile_pool(name="pps", bufs=1, space="PSUM"))
                cum_ps = pps.tile([P, NT], f32)
                nc.tensor.matmul(cum_ps[:], utri[:], msk[:])
                cum = gt.tile([P, NT], f32)
                nc.vector.tensor_copy(cum[:], cum_ps[:])
                pos_in = gt.tile([P, NT], f32)
                nc.vector.tensor_tensor(out=pos_in[:], in0=cum[:], in1=msk[:],
                                        op=ALU.subtract)

                tot_ps = pps.tile([1, NT], f32, name="tot_ps")
                nc.tensor.matmul(tot_ps[:], onescol[:], msk[:])
                tot = gt.tile([1, NT], f32)
                nc.vector.tensor_copy(tot[:], tot_ps[:])
                sc_a = gt.tile([1, NT], f32)
                nc.vector.tensor_copy(sc_a[:], tot[:])
                sc_b = gt.tile([1, NT], f32)
                for sh in (1, 2, 4, 8, 16):
                    nc.vector.tensor_copy(sc_b[:, 0:sh], sc_a[:, 0:sh])
                    nc.vector.tensor_tensor(out=sc_b[:, sh:NT],
                                            in0=sc_a[:, sh:NT],
                                            in1=sc_a[:, 0:NT - sh],
                                            op=ALU.add)
                    sc_a, sc_b = sc_b, sc_a
                offs_ex = gt.tile([1, NT], f32)
                nc.vector.tensor_tensor(out=offs_ex[:], in0=sc_a[:],
                                        in1=tot[:], op=ALU.subtract)

                offs_ps = pps.tile([P, NT], f32)
                nc.tensor.matmul(offs_ps[:], ones1[:], offs_ex[:])
                slot = gt.tile([P, NT], f32)
                nc.vector.tensor_tensor(out=slot[:], in0=pos_in[:],
                                        in1=offs_ps[:], op=ALU.add)

            # slot_s = slot*m + TRASH*(1-m)
            sm = gt.tile([P, NT], f32)
            nc.vector.tensor_tensor(out=sm[:], in0=slot[:], in1=msk[:],
                                    op=ALU.mult)
            minv = gt.tile([P, NT], f32)
            nc.vector.tensor_scalar(minv[:], msk[:], -float(TRASH),
                                    float(TRASH), ALU.mult, ALU.add)
            slot_s = gt.tile([P, NT], f32)
            nc.vector.tensor_tensor(out=slot_s[:], in0=sm[:], in1=minv[:],
                                    op=ALU.add)
            slot_i = gt.tile([P, NT], i32)
            nc.vector.tensor_copy(slot_i[:], slot_s[:])

            pack = gt.tile([P, NT, 8], f32)
            nc.vector.memset(pack[:], 0.0)
            nc.vector.tensor_copy(pack[:, :, 0], iotatok[:])
            nc.vector.tensor_copy(pack[:, :, 1], g_e[:])

            for t in range(NT):
                nc.gpsimd.indirect_dma_start(
                    out=scr_d[:],
                    out_offset=IndirectOffsetOnAxis(ap=slot_i[:, t:t + 1],
                                                    axis=0),
                    in_=pack[:, t, :], in_offset=None)

            # read back compacted (token_id, gate) list
            idxg_f = gt.tile([P, NTC, 2], f32)
            nc.sync.dma_start(
                idxg_f[:],
                scr_d[0:C_CAP, 0:2].rearrange("(t p) c -> p t c", p=P))
            idx_f = gt.tile([P, NTC], f32)
            nc.vector.tensor_copy(idx_f[:], idxg_f[:, :, 0])
            g_all = gt.tile([P, NTC], f32)
            nc.vector.tensor_copy(g_all[:], idxg_f[:, :, 1])
            idx_i = gt.tile([P, NTC], i32)
            nc.vector.tensor_copy(idx_i[:], idx_f[:])

            idx_gather = gt.tile([P, NTC], i32)
            nc.vector.tensor_scalar_max(idx_gather[:], idx_i[:], 0)
            mneg = gt.tile([P, NTC], i32)
            nc.vector.tensor_scalar(mneg[:], idx_i[:], 0, None, ALU.is_lt)
            nc.vector.tensor_scalar(mneg[:], mneg[:], N + 1, None, ALU.mult)
            idx_scat = gt.tile([P, NTC], i32)
            nc.vector.tensor_tensor(out=idx_scat[:], in0=idx_i[:],
                                    in1=mneg[:], op=ALU.add)

            # ================= expert FFN =================
            fx = ctx.enter_context(tc.tile_pool(name="fx", bufs=4))
            fbig = ctx.enter_context(tc.tile_pool(name="fbig", bufs=1))
            fw1 = ctx.enter_context(tc.tile_pool(name="fw1", bufs=3))
            fw2 = ctx.enter_context(tc.tile_pool(name="fw2", bufs=3))
            fy = ctx.enter_context(tc.tile_pool(name="fy", bufs=3))
            ptr = ctx.enter_context(tc.tile_pool(name="ptr", bufs=2,
                                                 space="PSUM"))
            pf1 = ctx.enter_context(tc.tile_pool(name="pf1", bufs=2,
                                                 space="PSUM"))
            pf2 = ctx.enter_context(tc.tile_pool(name="pf2", bufs=1,
                                                 space="PSUM"))

            # gather all capacity tiles, transpose to xeT [128, DT, C_CAP]
            xeT = fbig.tile([P, DT, C_CAP], f32r, name="xeT")
            for k in range(NTC):
                xt = fx.tile([P, D], f32, tag="xe", name=f"xe_{k}")
                nc.gpsimd.indirect_dma_start(
                    out=xt[:], out_offset=None, in_=x_d[:],
                    in_offset=IndirectOffsetOnAxis(
                        ap=idx_gather[:, k:k + 1], axis=0))
                for dt_i in range(DT):
                    tp = ptr.tile([P, P], f32, tag="tp", name="tp")
                    nc.tensor.transpose(
                        tp[:], xt[:, dt_i * P:(dt_i + 1) * P], ident[:])
                    nc.vector.tensor_copy(
                        xeT[:, dt_i, k * P:(k + 1) * P], tp[:])

            hT = fbig.tile([P, HTQ, C_CAP], f32r, name="hT")
            yT = fbig.tile([P, DT, C_CAP], f32, name="yT")

            for q in range(NQ):
                # FFN1 for this h-quarter
                for ht_l in range(HTQ):
                    ht = q * HTQ + ht_l
                    w1t = fw1.tile([P, DT, P], f32r, tag="w1t", name="w1t")
                    nc.sync.dma_start(w1t[:], w1_d[ht])
                    for co, cw in CHUNKS:
                        ps1 = pf1.tile([P, 512], f32, tag="ps1", name="ps1")
                        for dt_i in range(DT):
                            nc.tensor.matmul(ps1[:, 0:cw],
                                             w1t[:, dt_i, :],
                                             xeT[:, dt_i, co:co + cw],
                                             start=(dt_i == 0),
                                             stop=(dt_i == DT - 1))
                        nc.scalar.activation(hT[:, ht_l, co:co + cw],
                                             ps1[:, 0:cw], AF.Relu,
                                             bias=b1_sb[:, ht:ht + 1])

                # FFN2: one d-tile at a time, accumulate into yT (SBUF)
                for d_i in range(DT):
                    w2t = fw2.tile([P, HTQ, P], f32r, tag="w2t", name="w2t")
                    nc.sync.dma_start(w2t[:], w2_d[q, d_i])
                    ps2 = [pf2.tile([P, cw], f32, tag=f"ps2_{ci}",
                                    name=f"ps2_{q}_{d_i}_{ci}")
                           for ci, (co, cw) in enumerate(CHUNKS)]
                    for ht_l in range(HTQ):
                        for ci, (co, cw) in enumerate(CHUNKS):
                            nc.tensor.matmul(ps2[ci][:], w2t[:, ht_l, :],
                                             hT[:, ht_l, co:co + cw],
                                             start=(ht_l == 0),
                                             stop=(ht_l == HTQ - 1))
                    for ci, (co, cw) in enumerate(CHUNKS):
                        if q == 0:
                            nc.scalar.activation(yT[:, d_i, co:co + cw],
                                                 ps2[ci][:], AF.Identity,
                                                 bias=b2_sb[:, d_i:d_i + 1])
                        else:
                            nc.vector.tensor_tensor(
                                out=yT[:, d_i, co:co + cw],
                                in0=ps2[ci][:],
                                in1=yT[:, d_i, co:co + cw],
                                op=ALU.add)

            # transpose back, scale by gate, scatter
            for k in range(NTC):
                yk = fy.tile([P, D], f32, tag="yk", name=f"yk_{k}")
                for dt_i in range(DT):
                    tp2 = ptr.tile([P, P], f32, tag="tp", name="tp2")
                    nc.tensor.transpose(
                        tp2[:], yT[:, dt_i, k * P:(k + 1) * P], ident[:])
                    nc.vector.tensor_scalar_mul(
                        yk[:, dt_i * P:(dt_i + 1) * P], tp2[:],
                        g_all[:, k:k + 1])
                nc.gpsimd.indirect_dma_start(
                    out=out_d[:],
                    out_offset=IndirectOffsetOnAxis(
                        ap=idx_scat[:, k:k + 1], axis=0),
                    in_=yk[:], in_offset=None)

    nc.compile()
    _NC_CACHE[key] = nc
    return nc


def make_feeds(x, Wg, bg, W1, b1, W2, b2, e):
    """Per-core input map (host-side sharding + layout prep)."""
    xTt = np.ascontiguousarray(
        x.reshape(8, 512, DT, P).transpose(2, 0, 3, 1))
    w1t = np.ascontiguousarray(
        W1[e].reshape(DT, P, HT, P).transpose(2, 1, 0, 3))
    w2t = np.ascontiguousarray(
        W2[e].reshape(NQ, HTQ, P, DT, P).transpose(0, 3, 2, 1, 4))
    iota8 = np.tile(np.arange(8, dtype=np.float32)[None, :], (P, 1))
    iota8r = np.tile((7 - np.arange(8, dtype=np.float32))[None, :], (P, 1))
    iotatok = (np.arange(NT, dtype=np.float32)[None, :] * P
               + np.arange(P, dtype=np.float32)[:, None])
    fillrow = np.zeros((P, 8), np.float32)
    fillrow[:, 0] = -1.0
    return {
        "x": x, "xTt": xTt, "Wg": Wg,
        "bgc": np.ascontiguousarray(bg[:, None]),
        "W1t": w1t, "b1e": np.ascontiguousarray(b1[e][None, :]),
        "W2t": w2t, "b2e": np.ascontiguousarray(b2[e][None, :]),
        "eid": np.full((P, 1), e, np.float32),
        "iota8": iota8, "iota8r": iota8r, "iotatok": iotatok,
        "ident": np.eye(P, dtype=np.float32),
        "utri": np.triu(np.ones((P, P), np.float32)),
        "ones1": np.ones((1, P), np.float32),
        "onescol": np.ones((P, 1), np.float32),
        "fillrow": fillrow,
    }


def kernel(x, Wg, bg, W1, b1, W2, b2):
    x = np.ascontiguousarray(np.asarray(x, dtype=np.float32))
    Wg = np.ascontiguousarray(np.asarray(Wg, dtype=np.float32))
    bg = np.asarray(bg, dtype=np.float32)
    W1 = np.asarray(W1, dtype=np.float32)
    b1 = np.asarray(b1, dtype=np.float32)
    W2 = np.asarray(W2, dtype=np.float32)
    b2 = np.asarray(b2, dtype=np.float32)

    nc = build()
    core_ids = list(range(E))
    in_maps = [make_feeds(x, Wg, bg, W1, b1, W2, b2, e) for e in core_ids]

    kwargs = {}
    if TRACE:
        kwargs = dict(trace=True, trace_cores=core_ids)
    res = run_bass_kernel_spmd(nc, in_maps, core_ids, **kwargs)
    LAST["res"] = res
    out = np.zeros((N, D), np.float32)
    for e in core_ids:
        out += res.results[e]["ypart"][:N]
    return out
